# revision 18
# baseline (speedup 1.0000x reference)
"""GAT (3-layer, heads=1) + linear head on 8 Trainium2 NeuronCores — v8.

vs v5:
  - Correctness: exp is computed as exp(lrelu(z) - C) with C=6 baked into the
    Exp bias (softmax-invariant); layer-0 logits reach 11.35 which overflows
    f16 exp (limit 11.09) and NaN-cascaded in v5.
  - Member-contiguous window layout [m.G0|m.G1|m.self] via 4 gathers/group
    (per-gather fixed cost measured ~0); the self-loop row is copied into the
    window from agst8 so softmax/prod/reduce treat it like any edge.
  - Whole-group z/lrelu chains, per-member Exp with accum_out giving the full
    denominator directly; xn fused into one scalar_tensor_tensor; both prods
    issued before the reduces so the gather buffer frees early.
  - lrelu stays as abs+scalar_tensor_tensor (HW Lrelu ignores alpha); the
    reduce stays a contiguous add-tree (strided tensor_reduce measured 3.5x
    slower), hence use_reduce=False.
  - agst f16 staging dropped (epilogue writes agst8 f8 + es f16 only).
  - tabs allocated addr_space="Shared" (faster HBM-HBM AllGather).
Pool SWDGE desc-gen (7.4 ns/idx single-queue, HW-measured; elem-size-
independent, negative idxs not cheaper) dominated at ~5.3 ms total.  v10:
Bacc(num_swdge_queues=4) + round-robin queue_num parallelizes desc-gen across
Q7 cores -> 5.79 ms to 3.44 ms (the single biggest win of the session).
"""

from contextlib import ExitStack

import numpy as np

import concourse.bass as bass
import concourse.bacc as bacc
import concourse.mybir as mybir
import concourse.tile as tile
from concourse.bass_utils import run_bass_kernel_spmd
from concourse.masks import make_identity

P = 128
NC = 8
NEG_SLOPE = 0.2
C_SHIFT = 6.0
F16 = mybir.dt.float16
F32 = mybir.dt.float32
I16 = mybir.dt.int16
F8 = mybir.dt.float8e4
AF = mybir.ActivationFunctionType
ALU = mybir.AluOpType

N_FULL = 50000
H_DIM = 128
C_OUT = 40
ROW = 256          # fp8 elems per table row (256 B): [h f8 x128 | es f16]
AG = 132           # useful row bytes shipped to the collective


class Plan:
    def __init__(self, n, h, c_out, n_layers=3):
        self.n = n
        self.h = h
        self.c_out = c_out
        self.n_layers = n_layers
        self.shard = ((n + NC * P - 1) // (NC * P)) * P
        self.np_ = self.shard * NC
        self.t = self.shard // P
        self.w0 = self.shard * (NC // 2)
        assert self.w0 <= 32768 and self.np_ - self.w0 <= 32768
        self.groups = None


def _wrap_idx(flat):
    """int16 idx -> [128, len/16] SWDGE layout (16-partition wrap, replicated)."""
    flat = np.asarray(flat, dtype=np.int16)
    assert len(flat) % 16 == 0
    arr = flat.reshape(-1, 16).T
    return np.tile(arr, (8, 1))


def prep(plan: Plan, edge_index: np.ndarray):
    n, np_, shard, t = plan.n, plan.np_, plan.shard, plan.t
    # Self-loops (the appended arange) are handled locally, not gathered.
    src = edge_index[0].astype(np.int64)
    dst = edge_index[1].astype(np.int64)
    deg = np.bincount(dst, minlength=np_)

    # deal nodes to cores, snake in degree order -> balanced edge counts
    order = np.argsort(-deg, kind="stable")
    i = np.arange(np_)
    r = i % (2 * NC)
    core_of = np.empty(np_, dtype=np.int64)
    core_of[order] = np.where(r < NC, r, 2 * NC - 1 - r)

    src_is_w0 = core_of[src] < (NC // 2)
    d0 = np.bincount(dst[src_is_w0], minlength=np_)
    d1 = deg - d0

    # within-core sort: max(d0,d1) desc, boustrophedon on d0-d1 -> tight rank
    # groups of 128 (rank group k = ranks [k*128,(k+1)*128))
    rank_nodes = np.empty((NC, shard), dtype=np.int64)
    for c in range(NC):
        nodes = np.where(core_of == c)[0]
        m = np.maximum(d0[nodes], d1[nodes])
        s = d0[nodes] - d1[nodes] + 100
        key = m * 200000 + np.where(m % 2 == 0, s, 200 - s) * 100
        rank_nodes[c] = nodes[np.argsort(-key, kind="stable")]

    # per-rank-group window maxima (over cores & partitions)
    d0r = d0[rank_nodes].reshape(NC, t, P)
    d1r = d1[rank_nodes].reshape(NC, t, P)
    g0r = np.maximum(d0r.max(axis=(0, 2)), 1)
    g1r = np.maximum(d1r.max(axis=(0, 2)), 1)
    jr = g0r + g1r

    # group rank-groups into gather calls: biggest NSINGLE alone, rest paired
    rk = np.argsort(-jr, kind="stable")
    NSINGLE = 5
    groups_rg = [[int(rk[k])] for k in range(NSINGLE)]
    rest = rk[NSINGLE:]
    nr = len(rest)
    for k in range(nr // 2):
        groups_rg.append([int(rest[k]), int(rest[nr - 1 - k])])
    if nr % 2:
        groups_rg.append([int(rest[nr // 2])])
    # process big groups first
    groups_rg.sort(key=lambda mem: -sum(int(jr[r_]) for r_ in mem))

    # assign tile indices in processing order; tile ti <- rank group rg
    tile_of_rank = {}
    ti = 0
    for mem in groups_rg:
        for rg in mem:
            tile_of_rank[rg] = ti
            ti += 1
    assert ti == t

    # final node placement: tile ti of core c holds rank group rg's nodes
    new2old = np.empty(np_, dtype=np.int64)
    for c in range(NC):
        for rg, tix in tile_of_rank.items():
            new2old[c * shard + tix * P:(c * shard + (tix + 1) * P)] = \
                rank_nodes[c, rg * P:(rg + 1) * P]
    old2new = np.empty(np_, dtype=np.int64)
    old2new[new2old] = np.arange(np_)

    nsrc = old2new[src]
    ndst = old2new[dst]

    d0n = d0[new2old].reshape(NC, t, P)
    g0 = np.maximum(d0n.max(axis=(0, 2)), 1)
    g1 = np.maximum(d1[new2old].reshape(NC, t, P).max(axis=(0, 2)), 1)

    # group layout: [m0.G0 m1.G0 | m0.G1 m1.G1 | self_m0 self_m1]
    plan.groups = []
    for mem in groups_rg:
        tis = [tile_of_rank[rg] for rg in mem]
        G0g = int(sum(g0[x] for x in tis))
        G1g = int(sum(g1[x] for x in tis))
        members = []
        o0 = 0
        o1 = G0g
        for mi, x in enumerate(tis):
            members.append((int(x), int(g0[x]), int(g1[x]), o0, o1,
                            G0g + G1g + mi))
            o0 += int(g0[x])
            o1 += int(g1[x])
        plan.groups.append({"members": members, "W": G0g + G1g + len(tis),
                            "G0g": G0g, "G1g": G1g})
    plan.jgmax = max(g["W"] for g in plan.groups)
    plan.jtmax = max(m[1] + m[2] + 1 for g_ in plan.groups
                     for m in g_["members"])

    # edges sorted by (dst, window); each dst's w0 edges first
    eorder = np.argsort(ndst * 2 + (~src_is_w0).astype(np.int64), kind="stable")
    s_sorted = nsrc[eorder]
    counts = np.bincount(ndst, minlength=np_)
    starts = np.zeros(np_ + 1, dtype=np.int64)
    np.cumsum(counts, out=starts[1:])

    per_core = []
    total_slots = 0
    for c in range(NC):
        idx0_parts, idx1_parts, mask_parts = [], [], []
        for grp in plan.groups:
            mb = np.full((P, grp["W"]), -30000.0, dtype=np.float32)
            for (ti2, G0, G1, o0m, o1m, slm) in grp["members"]:
                a0 = np.zeros((G0, P), dtype=np.int16)
                a1 = np.zeros((G1, P), dtype=np.int16)
                for p in range(P):
                    node = c * shard + ti2 * P + p
                    s0, s1 = starts[node], starts[node + 1]
                    srcs = s_sorted[s0:s1]
                    k0 = int(d0n[c, ti2, p])
                    a0[:k0, p] = srcs[:k0]
                    a1[: s1 - s0 - k0, p] = srcs[k0:] - plan.w0
                    mb[p, o0m:o0m + k0] = 0.0
                    mb[p, o1m:o1m + (s1 - s0 - k0)] = 0.0
                mb[:, slm] = 0.0                     # self slot
                idx0_parts.append(_wrap_idx(a0.reshape(-1)))
                idx1_parts.append(_wrap_idx(a1.reshape(-1)))
                total_slots += (G0 + G1) * P
            mask_parts.append(mb)
        per_core.append({
            "idx0": np.concatenate(idx0_parts, axis=1),
            "idx1": np.concatenate(idx1_parts, axis=1),
            "maskb": np.ascontiguousarray(np.concatenate(mask_parts, axis=1)),
        })
    plan.slots = total_slots
    plan.l0 = per_core[0]["idx0"].shape[1]
    plan.l1 = per_core[0]["idx1"].shape[1]
    plan.lj = per_core[0]["maskb"].shape[1]
    return per_core, new2old


def build(plan: Plan, use_reduce=False):
    nc = bacc.Bacc(None, target_bir_lowering=False, num_swdge_queues=4)
    np_, shard, t, h, co = plan.np_, plan.shard, plan.t, plan.h, plan.c_out
    nl = plan.n_layers
    jgmax = plan.jgmax
    jtmax = plan.jtmax

    xT = nc.dram_tensor("xT", [P, shard], F16, kind="ExternalInput")
    idx0 = nc.dram_tensor("idx0", [P, plan.l0], I16, kind="ExternalInput")
    idx1 = nc.dram_tensor("idx1", [P, plan.l1], I16, kind="ExternalInput")
    maskb = nc.dram_tensor("maskb", [P, plan.lj], F32, kind="ExternalInput")
    Waugs = [nc.dram_tensor(f"Waug{l}", [h, h + 2], F16, kind="ExternalInput")
             for l in range(nl)]
    Bs = [nc.dram_tensor(f"B{l}", [P, h], F32, kind="ExternalInput")
          for l in range(nl)]
    Wo = nc.dram_tensor("Wo", [h, co], F16, kind="ExternalInput")
    bo = nc.dram_tensor("bo", [P, co], F32, kind="ExternalInput")
    out = nc.dram_tensor("out", [shard, co], F32, kind="ExternalOutput")

    tabs = [nc.dram_tensor(f"tab{l}", [np_, ROW], F8, kind="Internal",
                           addr_space="Shared")
            for l in range(nl)]
    agins = [nc.dram_tensor(f"agin{l}", [shard, ROW], F8, kind="Internal")
             for l in range(nl)]

    with tile.TileContext(nc) as tc, ExitStack() as ctx:
        const = ctx.enter_context(tc.tile_pool(name="const", bufs=1))
        gat = ctx.enter_context(tc.tile_pool(name="gat", bufs=5))
        pl = ctx.enter_context(tc.tile_pool(name="pl", bufs=2))
        pp = ctx.enter_context(tc.tile_pool(name="pp", bufs=2))
        psT = ctx.enter_context(tc.tile_pool(name="psT", bufs=2, space="PSUM"))
        psE = ctx.enter_context(tc.tile_pool(name="psE", bufs=2, space="PSUM"))

        ident = const.tile([P, P], F16, tag="ident")
        make_identity(nc, ident[:])
        xT_sb = const.tile([P, shard], F16, tag="xT")
        idx0_sb = const.tile([P, plan.l0], I16, tag="idx0")
        idx1_sb = const.tile([P, plan.l1], I16, tag="idx1")
        maskb_sb = const.tile([P, plan.lj], F32, tag="maskb")
        nc.sync.dma_start(xT_sb[:], xT[:])
        nc.sync.dma_start(idx0_sb[:], idx0[:])
        nc.sync.dma_start(idx1_sb[:], idx1[:])
        nc.sync.dma_start(maskb_sb[:], maskb[:])
        Waug_sb = [const.tile([h, h + 2], F16, tag=f"Waug{l}",
                              name=f"Waug_sb{l}") for l in range(nl)]
        B_sb = [const.tile([P, h], F32, tag=f"B{l}", name=f"B_sb{l}")
                for l in range(nl)]
        for l in range(nl):
            nc.sync.dma_start(Waug_sb[l][:], Waugs[l][:])
            nc.sync.dma_start(B_sb[l][:], Bs[l][:])
        Wo_sb = const.tile([h, co], F16, tag="Wo")
        bo_sb = const.tile([P, co], F32, tag="bo")
        nc.sync.dma_start(Wo_sb[:], Wo[:])
        nc.sync.dma_start(bo_sb[:], bo[:])
        agst8 = const.tile([P, t, AG], F8, tag="agst8")
        agst8_16 = agst8[:].bitcast(F16)
        negC = const.tile([P, 1], F32, tag="negC")
        nc.vector.memset(negC[:], -C_SHIFT)
        ed_sb = [const.tile([P, t], F32, tag=f"ed{l}", name=f"ed_sb{l}")
                 for l in range(nl)]

        def ship(l):
            """DMA agst8 to agin[l] and AllGather into tab[l]."""
            dst = agins[l][:, 0:AG].rearrange("(g p) f -> p g f", p=P)
            nc.sync.dma_start(dst, agst8[:, :, :])
            nc.gpsimd.collective_compute(
                "AllGather", ALU.bypass, replica_groups=[list(range(NC))],
                ins=[agins[l][:, :]], outs=[tabs[l][:, :]])

        def epilogue(l, ti, ps):
            """Stage next-layer row pieces from psE ps = xn16 @ Waug[l]."""
            nc.scalar.copy(ed_sb[l][:, ti:ti + 1], ps[:, h + 1:h + 2])
            nc.scalar.copy(agst8[:, ti, 0:h], ps[:, 0:h])
            nc.scalar.copy(agst8_16[:, ti, h // 2:h // 2 + 1], ps[:, h:h + 1])

        # ---- layer-0 own-shard rows: x @ [W0 | W0 a_s | W0 a_d] ------------
        for ti in range(t):
            ps = psE.tile([P, h + 2], F32, tag="psA")
            nc.tensor.matmul(ps[:], xT_sb[:, ti * P:(ti + 1) * P], Waug_sb[0][:])
            epilogue(0, ti, ps)
        ship(0)

        qn = 0
        for l in range(nl):
            table = tabs[l]
            og = o0 = o1 = 0
            for grp in plan.groups:
                W = grp["W"]
                G0g, G1g = grp["G0g"], grp["G1g"]
                g = gat.tile([P, jgmax, ROW], F8, tag="g")
                g16 = g[:].bitcast(F16)
                nc.gpsimd.dma_gather(
                    g[:, 0:G0g, :], table[0:plan.w0, :],
                    idx0_sb[:, o0:o0 + G0g * 8], G0g * P, G0g * P, ROW,
                    single_packet=False, queue_num=qn % 4)
                nc.gpsimd.dma_gather(
                    g[:, G0g:G0g + G1g, :], table[plan.w0:np_, :],
                    idx1_sb[:, o1:o1 + G1g * 8], G1g * P, G1g * P, ROW,
                    single_packet=False, queue_num=(qn + 1) % 4)
                qn += 2
                o0 += G0g * 8
                o1 += G1g * 8
                for (ti, G0, G1, o0m, o1m, slm) in grp["members"]:
                    # self-loop row from the local stage (not gathered)
                    nc.scalar.copy(g[:, slm:slm + 1, 0:AG],
                                   agst8[:, ti:ti + 1, :])

                edb = pl.tile([P, jgmax], F32, tag="edb")
                z = pl.tile([P, jgmax], F32, tag="z")
                za = pl.tile([P, jgmax], F32, tag="za")
                lg = pl.tile([P, jgmax], F32, tag="lg")
                w16 = pl.tile([P, jgmax], F16, tag="w16")
                den3 = pl.tile([P, 6], F32, tag="den3")
                den = pl.tile([P, 2], F32, tag="den")
                for mi, (ti, G0, G1, o0m, o1m, slm) in enumerate(
                        grp["members"]):
                    bias = ed_sb[l][:, ti:ti + 1]
                    for (a, n_) in ((o0m, G0), (o1m, G1), (slm, 1)):
                        nc.scalar.activation(
                            edb[:, a:a + n_], maskb_sb[:, og + a:og + a + n_],
                            AF.Identity, bias=bias, scale=1.0)
                nc.vector.tensor_add(
                    z[:, 0:W],
                    g16[:, 0:W, h // 2:h // 2 + 1].rearrange(
                        "p j one -> p (j one)"),
                    edb[:, 0:W])
                # lrelu(z) = (1+a)/2*z + (1-a)/2*|z|  (HW Lrelu ignores alpha)
                nc.scalar.activation(za[:, 0:W], z[:, 0:W], AF.Abs,
                                     scale=(1 - NEG_SLOPE) / 2)
                nc.vector.scalar_tensor_tensor(
                    lg[:, 0:W], z[:, 0:W], (1 + NEG_SLOPE) / 2, za[:, 0:W],
                    op0=ALU.mult, op1=ALU.add)
                nm = len(grp["members"])
                for mi, (ti, G0, G1, o0m, o1m, slm) in enumerate(
                        grp["members"]):
                    for k, (a, n_) in enumerate(
                            ((o0m, G0), (o1m, G1), (slm, 1))):
                        nc.scalar.activation(
                            w16[:, a:a + n_], lg[:, a:a + n_], AF.Exp,
                            bias=negC[:, 0:1],
                            accum_out=den3[:, k * nm + mi:k * nm + mi + 1])
                nc.vector.tensor_add(den[:, 0:nm], den3[:, 0:nm],
                                     den3[:, nm:2 * nm])
                nc.vector.tensor_add(den[:, 0:nm], den[:, 0:nm],
                                     den3[:, 2 * nm:3 * nm])
                rcpg = pl.tile([P, 2], F32, tag="rcpg")
                nc.vector.reciprocal(rcpg[:, 0:nm], den[:, 0:nm])
                prods = []
                for mi, (ti, G0, G1, o0m, o1m, slm) in enumerate(
                        grp["members"]):
                    J = G0 + G1 + 1
                    prod = pp.tile([P, jtmax, h], F16, tag="prod")
                    for (dsta, a, n_) in ((0, o0m, G0), (G0, o1m, G1),
                                          (G0 + G1, slm, 1)):
                        nc.vector.tensor_mul(
                            prod[:, dsta:dsta + n_, :], g[:, a:a + n_, 0:h],
                            w16[:, a:a + n_].unsqueeze(2).to_broadcast(
                                [P, n_, h]))
                    prods.append(prod)
                for mi, (ti, G0, G1, o0m, o1m, slm) in enumerate(
                        grp["members"]):
                    J = G0 + G1 + 1
                    prod = prods[mi]
                    numA = pl.tile([P, h], F32, tag="numA")
                    if use_reduce:
                        nc.vector.tensor_reduce(
                            numA[:, :], prod[:, 0:J, :].rearrange(
                                "p j f -> p f j"),
                            axis=mybir.AxisListType.X, op=ALU.add)
                    else:
                        cur = J
                        while cur > 2:
                            half = cur // 2
                            nc.vector.tensor_add(
                                prod[:, 0:half, :], prod[:, 0:half, :],
                                prod[:, half:2 * half, :])
                            if cur % 2:
                                nc.vector.tensor_add(
                                    prod[:, 0:1, :], prod[:, 0:1, :],
                                    prod[:, cur - 1:cur, :])
                            cur = half
                        if cur == 2:
                            nc.vector.tensor_add(numA[:, :].unsqueeze(1),
                                                 prod[:, 0:1, :],
                                                 prod[:, 1:2, :])
                        else:
                            nc.vector.tensor_copy(numA[:, :].unsqueeze(1),
                                                  prod[:, 0:1, :])
                    xn = pl.tile([P, h], F32, tag="xn")
                    nc.vector.scalar_tensor_tensor(
                        xn[:], numA[:], rcpg[:, mi:mi + 1], B_sb[l][:, :],
                        op0=ALU.mult, op1=ALU.add)
                    xn16 = pl.tile([P, h], F16, tag="xn16")
                    nc.scalar.activation(xn16[:], xn[:], AF.Relu)
                    tp = psT.tile([P, P], F16, tag="tp")
                    nc.tensor.transpose(tp[:], xn16[:], ident[:])
                    xnT = pl.tile([P, h], F16, tag="xnT")
                    nc.scalar.copy(xnT[:], tp[:])
                    if l < nl - 1:
                        ps = psE.tile([P, h + 2], F32, tag="psA")
                        nc.tensor.matmul(ps[:], xnT[:], Waug_sb[l + 1][:])
                        epilogue(l + 1, ti, ps)
                    else:
                        ops = psE.tile([P, co], F32, tag="ops")
                        nc.tensor.matmul(ops[:], xnT[:], Wo_sb[:])
                        ot = pl.tile([P, co], F32, tag="ot")
                        nc.vector.tensor_add(ot[:], ops[:], bo_sb[:, :])
                        nc.sync.dma_start(out[ti * P:(ti + 1) * P, :], ot[:])
                og += W
            if l < nl - 1:
                ship(l + 1)

    nc.compile()
    return nc


def _make_in_maps(plan, per_core, new2old, inputs):
    n, np_, shard, h = plan.n, plan.np_, plan.shard, plan.h
    xsrc = np.asarray(inputs["x"], dtype=np.float32)
    xp = np.zeros((np_, h), dtype=np.float32)
    valid = new2old < n
    xp[valid] = xsrc[new2old[valid]]

    base = {
        "Wo": np.asarray(inputs["Wo"], np.float16),
        "bo": np.tile(np.asarray(inputs["bo"], np.float32).reshape(1, -1),
                      (P, 1)),
    }
    for l in range(plan.n_layers):
        W = np.asarray(inputs[f"W{l}"], np.float32)
        a_s = np.asarray(inputs[f"as{l}"], np.float32)
        a_d = np.asarray(inputs[f"ad{l}"], np.float32)
        Waug = np.concatenate([W, (W @ a_s)[:, None], (W @ a_d)[:, None]],
                              axis=1)
        base[f"Waug{l}"] = Waug.astype(np.float16)
        base[f"B{l}"] = np.tile(
            np.asarray(inputs[f"b{l}"], np.float32).reshape(1, -1), (P, 1))
    in_maps = []
    for c in range(NC):
        m = dict(base)
        xcs = xp[c * shard:(c + 1) * shard]
        m["xT"] = np.ascontiguousarray(xcs.T.astype(np.float16))
        m.update(per_core[c])
        in_maps.append(m)
    return in_maps


_CACHE = {}


def run_gat(inputs, n, h, c_out, **spmd_kwargs):
    edge_index = np.asarray(inputs["edge_index"])
    key = (n, h, c_out, edge_index.shape[1])
    if key not in _CACHE:
        plan = Plan(n, h, c_out)
        per_core, new2old = prep(plan, edge_index)
        nc = build(plan)
        _CACHE[key] = (plan, per_core, new2old, nc)
    plan, per_core, new2old, nc = _CACHE[key]

    in_maps = _make_in_maps(plan, per_core, new2old, inputs)
    res = run_bass_kernel_spmd(nc, in_maps, core_ids=list(range(NC)),
                               **spmd_kwargs)
    shards = [res.results[c]["out"] for c in range(NC)]
    full = np.concatenate(shards, axis=0)
    outp = np.empty((plan.n, plan.c_out), dtype=np.float32)
    valid = new2old < plan.n
    outp[new2old[valid]] = full[valid]
    return outp, res


def kernel(**inputs) -> np.ndarray:
    outp, _ = run_gat(inputs, N_FULL, H_DIM, C_OUT)
    return outp


# revision 20
# speedup vs baseline: 1.2549x; 1.2549x over previous
"""GAT (3-layer, heads=1) + linear head on 8 Trainium2 NeuronCores — v8.

vs v5:
  - Correctness: exp is computed as exp(lrelu(z) - C) with C=6 baked into the
    Exp bias (softmax-invariant); layer-0 logits reach 11.35 which overflows
    f16 exp (limit 11.09) and NaN-cascaded in v5.
  - Member-contiguous window layout [m.G0|m.G1|m.self] via 4 gathers/group
    (per-gather fixed cost measured ~0); the self-loop row is copied into the
    window from agst8 so softmax/prod/reduce treat it like any edge.
  - Whole-group z/lrelu chains, per-member Exp with accum_out giving the full
    denominator directly; xn fused into one scalar_tensor_tensor; both prods
    issued before the reduces so the gather buffer frees early.
  - lrelu stays as abs+scalar_tensor_tensor (HW Lrelu ignores alpha); the
    reduce stays a contiguous add-tree (strided tensor_reduce measured 3.5x
    slower), hence use_reduce=False.
  - agst f16 staging dropped (epilogue writes agst8 f8 + es f16 only).
  - tabs allocated addr_space="Shared" (faster HBM-HBM AllGather).
Pool SWDGE desc-gen (7.4 ns/idx, HW-measured; elem-size-independent, queue 0
only, negative idxs not cheaper) dominates: ~1.66 ms/layer is the floor.
"""

from contextlib import ExitStack

import numpy as np

import concourse.bass as bass
import concourse.bacc as bacc
import concourse.mybir as mybir
import concourse.tile as tile
from concourse.bass_utils import run_bass_kernel_spmd
from concourse.masks import make_identity

P = 128
NC = 8
NEG_SLOPE = 0.2
C_SHIFT = 6.0
F16 = mybir.dt.float16
F32 = mybir.dt.float32
I16 = mybir.dt.int16
F8 = mybir.dt.float8e4
AF = mybir.ActivationFunctionType
ALU = mybir.AluOpType

N_FULL = 50000
H_DIM = 128
C_OUT = 40
ROW = 256          # fp8 elems per table row (256 B): [h f8 x128 | es f16]
AG = 132           # useful row bytes shipped to the collective


class Plan:
    def __init__(self, n, h, c_out, n_layers=3):
        self.n = n
        self.h = h
        self.c_out = c_out
        self.n_layers = n_layers
        self.shard = ((n + NC * P - 1) // (NC * P)) * P
        self.np_ = self.shard * NC
        self.t = self.shard // P
        self.w0 = self.shard * (NC // 2)
        assert self.w0 <= 32768 and self.np_ - self.w0 <= 32768
        self.groups = None


def _wrap_idx(flat):
    """int16 idx -> [128, len/16] SWDGE layout (16-partition wrap, replicated)."""
    flat = np.asarray(flat, dtype=np.int16)
    assert len(flat) % 16 == 0
    arr = flat.reshape(-1, 16).T
    return np.tile(arr, (8, 1))


def prep(plan: Plan, edge_index: np.ndarray):
    n, np_, shard, t = plan.n, plan.np_, plan.shard, plan.t
    # Self-loops (the appended arange) are handled locally, not gathered.
    src = edge_index[0].astype(np.int64)
    dst = edge_index[1].astype(np.int64)
    deg = np.bincount(dst, minlength=np_)

    # deal nodes to cores, snake in degree order -> balanced edge counts
    order = np.argsort(-deg, kind="stable")
    i = np.arange(np_)
    r = i % (2 * NC)
    core_of = np.empty(np_, dtype=np.int64)
    core_of[order] = np.where(r < NC, r, 2 * NC - 1 - r)

    src_is_w0 = core_of[src] < (NC // 2)
    d0 = np.bincount(dst[src_is_w0], minlength=np_)
    d1 = deg - d0

    # within-core sort: max(d0,d1) desc, boustrophedon on d0-d1 -> tight rank
    # groups of 128 (rank group k = ranks [k*128,(k+1)*128))
    rank_nodes = np.empty((NC, shard), dtype=np.int64)
    for c in range(NC):
        nodes = np.where(core_of == c)[0]
        m = np.maximum(d0[nodes], d1[nodes])
        s = d0[nodes] - d1[nodes] + 100
        key = m * 200000 + np.where(m % 2 == 0, s, 200 - s) * 100
        rank_nodes[c] = nodes[np.argsort(-key, kind="stable")]

    # per-rank-group window maxima (over cores & partitions)
    d0r = d0[rank_nodes].reshape(NC, t, P)
    d1r = d1[rank_nodes].reshape(NC, t, P)
    g0r = np.maximum(d0r.max(axis=(0, 2)), 1)
    g1r = np.maximum(d1r.max(axis=(0, 2)), 1)
    jr = g0r + g1r

    # group rank-groups into gather calls: biggest NSINGLE alone, rest paired
    rk = np.argsort(-jr, kind="stable")
    NSINGLE = 5
    groups_rg = [[int(rk[k])] for k in range(NSINGLE)]
    rest = rk[NSINGLE:]
    nr = len(rest)
    for k in range(nr // 2):
        groups_rg.append([int(rest[k]), int(rest[nr - 1 - k])])
    if nr % 2:
        groups_rg.append([int(rest[nr // 2])])
    # process big groups first
    groups_rg.sort(key=lambda mem: -sum(int(jr[r_]) for r_ in mem))

    # assign tile indices in processing order; tile ti <- rank group rg
    tile_of_rank = {}
    ti = 0
    for mem in groups_rg:
        for rg in mem:
            tile_of_rank[rg] = ti
            ti += 1
    assert ti == t

    # final node placement: tile ti of core c holds rank group rg's nodes
    new2old = np.empty(np_, dtype=np.int64)
    for c in range(NC):
        for rg, tix in tile_of_rank.items():
            new2old[c * shard + tix * P:(c * shard + (tix + 1) * P)] = \
                rank_nodes[c, rg * P:(rg + 1) * P]
    old2new = np.empty(np_, dtype=np.int64)
    old2new[new2old] = np.arange(np_)

    nsrc = old2new[src]
    ndst = old2new[dst]

    d0n = d0[new2old].reshape(NC, t, P)
    g0 = np.maximum(d0n.max(axis=(0, 2)), 1)
    g1 = np.maximum(d1[new2old].reshape(NC, t, P).max(axis=(0, 2)), 1)

    # group layout: [m0.G0 m1.G0 | m0.G1 m1.G1 | self_m0 self_m1]
    plan.groups = []
    for mem in groups_rg:
        tis = [tile_of_rank[rg] for rg in mem]
        G0g = int(sum(g0[x] for x in tis))
        G1g = int(sum(g1[x] for x in tis))
        members = []
        o0 = 0
        o1 = G0g
        for mi, x in enumerate(tis):
            members.append((int(x), int(g0[x]), int(g1[x]), o0, o1,
                            G0g + G1g + mi))
            o0 += int(g0[x])
            o1 += int(g1[x])
        plan.groups.append({"members": members, "W": G0g + G1g + len(tis),
                            "G0g": G0g, "G1g": G1g})
    plan.jgmax = max(g["W"] for g in plan.groups)
    plan.jtmax = max(m[1] + m[2] + 1 for g_ in plan.groups
                     for m in g_["members"])

    # edges sorted by (dst, window); each dst's w0 edges first
    eorder = np.argsort(ndst * 2 + (~src_is_w0).astype(np.int64), kind="stable")
    s_sorted = nsrc[eorder]
    counts = np.bincount(ndst, minlength=np_)
    starts = np.zeros(np_ + 1, dtype=np.int64)
    np.cumsum(counts, out=starts[1:])

    per_core = []
    total_slots = 0
    for c in range(NC):
        idx0_parts, idx1_parts, mask_parts = [], [], []
        for grp in plan.groups:
            mb = np.full((P, grp["W"]), -30000.0, dtype=np.float32)
            for (ti2, G0, G1, o0m, o1m, slm) in grp["members"]:
                a0 = np.zeros((G0, P), dtype=np.int16)
                a1 = np.zeros((G1, P), dtype=np.int16)
                for p in range(P):
                    node = c * shard + ti2 * P + p
                    s0, s1 = starts[node], starts[node + 1]
                    srcs = s_sorted[s0:s1]
                    k0 = int(d0n[c, ti2, p])
                    a0[:k0, p] = srcs[:k0]
                    a1[: s1 - s0 - k0, p] = srcs[k0:] - plan.w0
                    mb[p, o0m:o0m + k0] = 0.0
                    mb[p, o1m:o1m + (s1 - s0 - k0)] = 0.0
                mb[:, slm] = 0.0                     # self slot
                idx0_parts.append(_wrap_idx(a0.reshape(-1)))
                idx1_parts.append(_wrap_idx(a1.reshape(-1)))
                total_slots += (G0 + G1) * P
            mask_parts.append(mb)
        per_core.append({
            "idx0": np.concatenate(idx0_parts, axis=1),
            "idx1": np.concatenate(idx1_parts, axis=1),
            "maskb": np.ascontiguousarray(np.concatenate(mask_parts, axis=1)),
        })
    plan.slots = total_slots
    plan.l0 = per_core[0]["idx0"].shape[1]
    plan.l1 = per_core[0]["idx1"].shape[1]
    plan.lj = per_core[0]["maskb"].shape[1]
    return per_core, new2old


def build(plan: Plan, use_reduce=False):
    nc = bacc.Bacc(None, target_bir_lowering=False, num_swdge_queues=4)
    np_, shard, t, h, co = plan.np_, plan.shard, plan.t, plan.h, plan.c_out
    nl = plan.n_layers
    jgmax = plan.jgmax
    jtmax = plan.jtmax

    xT = nc.dram_tensor("xT", [P, shard], F16, kind="ExternalInput")
    idx0 = nc.dram_tensor("idx0", [P, plan.l0], I16, kind="ExternalInput")
    idx1 = nc.dram_tensor("idx1", [P, plan.l1], I16, kind="ExternalInput")
    maskb = nc.dram_tensor("maskb", [P, plan.lj], F32, kind="ExternalInput")
    Waugs = [nc.dram_tensor(f"Waug{l}", [h, h + 2], F16, kind="ExternalInput")
             for l in range(nl)]
    Bs = [nc.dram_tensor(f"B{l}", [P, h], F32, kind="ExternalInput")
          for l in range(nl)]
    Wo = nc.dram_tensor("Wo", [h, co], F16, kind="ExternalInput")
    bo = nc.dram_tensor("bo", [P, co], F32, kind="ExternalInput")
    out = nc.dram_tensor("out", [shard, co], F32, kind="ExternalOutput")

    tabs = [nc.dram_tensor(f"tab{l}", [np_, ROW], F8, kind="Internal",
                           addr_space="Shared")
            for l in range(nl)]
    agins = [nc.dram_tensor(f"agin{l}", [shard, ROW], F8, kind="Internal")
             for l in range(nl)]

    with tile.TileContext(nc) as tc, ExitStack() as ctx:
        const = ctx.enter_context(tc.tile_pool(name="const", bufs=1))
        gat = ctx.enter_context(tc.tile_pool(name="gat", bufs=5))
        pl = ctx.enter_context(tc.tile_pool(name="pl", bufs=2))
        pp = ctx.enter_context(tc.tile_pool(name="pp", bufs=2))
        psT = ctx.enter_context(tc.tile_pool(name="psT", bufs=2, space="PSUM"))
        psE = ctx.enter_context(tc.tile_pool(name="psE", bufs=2, space="PSUM"))

        ident = const.tile([P, P], F16, tag="ident")
        make_identity(nc, ident[:])
        xT_sb = const.tile([P, shard], F16, tag="xT")
        idx0_sb = const.tile([P, plan.l0], I16, tag="idx0")
        idx1_sb = const.tile([P, plan.l1], I16, tag="idx1")
        maskb_sb = const.tile([P, plan.lj], F32, tag="maskb")
        nc.sync.dma_start(xT_sb[:], xT[:])
        nc.sync.dma_start(idx0_sb[:], idx0[:])
        nc.sync.dma_start(idx1_sb[:], idx1[:])
        nc.sync.dma_start(maskb_sb[:], maskb[:])
        Waug_sb = [const.tile([h, h + 2], F16, tag=f"Waug{l}",
                              name=f"Waug_sb{l}") for l in range(nl)]
        B_sb = [const.tile([P, h], F32, tag=f"B{l}", name=f"B_sb{l}")
                for l in range(nl)]
        for l in range(nl):
            nc.sync.dma_start(Waug_sb[l][:], Waugs[l][:])
            nc.sync.dma_start(B_sb[l][:], Bs[l][:])
        Wo_sb = const.tile([h, co], F16, tag="Wo")
        bo_sb = const.tile([P, co], F32, tag="bo")
        nc.sync.dma_start(Wo_sb[:], Wo[:])
        nc.sync.dma_start(bo_sb[:], bo[:])
        agst8 = const.tile([P, t, AG], F8, tag="agst8")
        agst8_16 = agst8[:].bitcast(F16)
        negC = const.tile([P, 1], F32, tag="negC")
        nc.vector.memset(negC[:], -C_SHIFT)
        ed_sb = [const.tile([P, t], F32, tag=f"ed{l}", name=f"ed_sb{l}")
                 for l in range(nl)]

        def ship(l):
            """DMA agst8 to agin[l] and AllGather into tab[l]."""
            dst = agins[l][:, 0:AG].rearrange("(g p) f -> p g f", p=P)
            nc.sync.dma_start(dst, agst8[:, :, :])
            nc.gpsimd.collective_compute(
                "AllGather", ALU.bypass, replica_groups=[list(range(NC))],
                ins=[agins[l][:, :]], outs=[tabs[l][:, :]])

        def epilogue(l, ti, ps):
            """Stage next-layer row pieces from psE ps = xn16 @ Waug[l]."""
            nc.scalar.copy(ed_sb[l][:, ti:ti + 1], ps[:, h + 1:h + 2])
            nc.scalar.copy(agst8[:, ti, 0:h], ps[:, 0:h])
            nc.scalar.copy(agst8_16[:, ti, h // 2:h // 2 + 1], ps[:, h:h + 1])

        # ---- layer-0 own-shard rows: x @ [W0 | W0 a_s | W0 a_d] ------------
        for ti in range(t):
            ps = psE.tile([P, h + 2], F32, tag="psA")
            nc.tensor.matmul(ps[:], xT_sb[:, ti * P:(ti + 1) * P], Waug_sb[0][:])
            epilogue(0, ti, ps)
        ship(0)

        qn = 0
        for l in range(nl):
            table = tabs[l]
            og = o0 = o1 = 0
            for grp in plan.groups:
                W = grp["W"]
                G0g, G1g = grp["G0g"], grp["G1g"]
                g = gat.tile([P, jgmax, ROW], F8, tag="g")
                g16 = g[:].bitcast(F16)
                for (ti, G0, G1, o0m, o1m, slm) in grp["members"]:
                    nc.gpsimd.dma_gather(
                        g[:, o0m:o0m + G0, :], table[0:plan.w0, :],
                        idx0_sb[:, o0:o0 + G0 * 8], G0 * P, G0 * P, ROW,
                        single_packet=False, queue_num=qn % 4)
                    nc.gpsimd.dma_gather(
                        g[:, o1m:o1m + G1, :], table[plan.w0:np_, :],
                        idx1_sb[:, o1:o1 + G1 * 8], G1 * P, G1 * P, ROW,
                        single_packet=False, queue_num=(qn + 1) % 4)
                    qn += 2
                    o0 += G0 * 8
                    o1 += G1 * 8
                for (ti, G0, G1, o0m, o1m, slm) in grp["members"]:
                    # self-loop row from the local stage (not gathered)
                    nc.scalar.copy(g[:, slm:slm + 1, 0:AG],
                                   agst8[:, ti:ti + 1, :])

                edb = pl.tile([P, jgmax], F32, tag="edb")
                z = pl.tile([P, jgmax], F32, tag="z")
                za = pl.tile([P, jgmax], F32, tag="za")
                lg = pl.tile([P, jgmax], F32, tag="lg")
                w16 = pl.tile([P, jgmax], F16, tag="w16")
                den3 = pl.tile([P, 6], F32, tag="den3")
                den = pl.tile([P, 2], F32, tag="den")
                for mi, (ti, G0, G1, o0m, o1m, slm) in enumerate(
                        grp["members"]):
                    bias = ed_sb[l][:, ti:ti + 1]
                    for (a, n_) in ((o0m, G0), (o1m, G1), (slm, 1)):
                        nc.scalar.activation(
                            edb[:, a:a + n_], maskb_sb[:, og + a:og + a + n_],
                            AF.Identity, bias=bias, scale=1.0)
                nc.vector.tensor_add(
                    z[:, 0:W],
                    g16[:, 0:W, h // 2:h // 2 + 1].rearrange(
                        "p j one -> p (j one)"),
                    edb[:, 0:W])
                # lrelu(z) = (1+a)/2*z + (1-a)/2*|z|  (HW Lrelu ignores alpha)
                nc.scalar.activation(za[:, 0:W], z[:, 0:W], AF.Abs,
                                     scale=(1 - NEG_SLOPE) / 2)
                nc.vector.scalar_tensor_tensor(
                    lg[:, 0:W], z[:, 0:W], (1 + NEG_SLOPE) / 2, za[:, 0:W],
                    op0=ALU.mult, op1=ALU.add)
                for mi, (ti, G0, G1, o0m, o1m, slm) in enumerate(
                        grp["members"]):
                    for k, (a, n_) in enumerate(
                            ((o0m, G0), (o1m, G1), (slm, 1))):
                        nc.scalar.activation(
                            w16[:, a:a + n_], lg[:, a:a + n_], AF.Exp,
                            bias=negC[:, 0:1],
                            accum_out=den3[:, 3 * mi + k:3 * mi + k + 1])
                    nc.vector.tensor_add(den[:, mi:mi + 1],
                                         den3[:, 3 * mi:3 * mi + 1],
                                         den3[:, 3 * mi + 1:3 * mi + 2])
                    nc.vector.tensor_add(den[:, mi:mi + 1], den[:, mi:mi + 1],
                                         den3[:, 3 * mi + 2:3 * mi + 3])
                prods = []
                for mi, (ti, G0, G1, o0m, o1m, slm) in enumerate(
                        grp["members"]):
                    J = G0 + G1 + 1
                    prod = pp.tile([P, jtmax, h], F16, tag="prod")
                    for (dsta, a, n_) in ((0, o0m, G0), (G0, o1m, G1),
                                          (G0 + G1, slm, 1)):
                        nc.vector.tensor_mul(
                            prod[:, dsta:dsta + n_, :], g[:, a:a + n_, 0:h],
                            w16[:, a:a + n_].unsqueeze(2).to_broadcast(
                                [P, n_, h]))
                    prods.append(prod)
                for mi, (ti, G0, G1, o0m, o1m, slm) in enumerate(
                        grp["members"]):
                    J = G0 + G1 + 1
                    prod = prods[mi]
                    numA = pl.tile([P, h], F32, tag="numA")
                    if use_reduce:
                        nc.vector.tensor_reduce(
                            numA[:, :], prod[:, 0:J, :].rearrange(
                                "p j f -> p f j"),
                            axis=mybir.AxisListType.X, op=ALU.add)
                    else:
                        cur = J
                        while cur > 2:
                            half = cur // 2
                            nc.vector.tensor_add(
                                prod[:, 0:half, :], prod[:, 0:half, :],
                                prod[:, half:2 * half, :])
                            if cur % 2:
                                nc.vector.tensor_add(
                                    prod[:, 0:1, :], prod[:, 0:1, :],
                                    prod[:, cur - 1:cur, :])
                            cur = half
                        if cur == 2:
                            nc.vector.tensor_add(numA[:, :].unsqueeze(1),
                                                 prod[:, 0:1, :],
                                                 prod[:, 1:2, :])
                        else:
                            nc.vector.tensor_copy(numA[:, :].unsqueeze(1),
                                                  prod[:, 0:1, :])
                    rcp = pl.tile([P, 1], F32, tag="rcp")
                    nc.vector.reciprocal(rcp[:], den[:, mi:mi + 1])
                    xn = pl.tile([P, h], F32, tag="xn")
                    nc.vector.scalar_tensor_tensor(
                        xn[:], numA[:], rcp[:, 0:1], B_sb[l][:, :],
                        op0=ALU.mult, op1=ALU.add)
                    xn16 = pl.tile([P, h], F16, tag="xn16")
                    nc.scalar.activation(xn16[:], xn[:], AF.Relu)
                    tp = psT.tile([P, P], F16, tag="tp")
                    nc.tensor.transpose(tp[:], xn16[:], ident[:])
                    xnT = pl.tile([P, h], F16, tag="xnT")
                    nc.scalar.copy(xnT[:], tp[:])
                    if l < nl - 1:
                        ps = psE.tile([P, h + 2], F32, tag="psA")
                        nc.tensor.matmul(ps[:], xnT[:], Waug_sb[l + 1][:])
                        epilogue(l + 1, ti, ps)
                    else:
                        ops = psE.tile([P, co], F32, tag="ops")
                        nc.tensor.matmul(ops[:], xnT[:], Wo_sb[:])
                        ot = pl.tile([P, co], F32, tag="ot")
                        nc.vector.tensor_add(ot[:], ops[:], bo_sb[:, :])
                        nc.sync.dma_start(out[ti * P:(ti + 1) * P, :], ot[:])
                og += W
            if l < nl - 1:
                ship(l + 1)

    nc.compile()
    return nc


def _make_in_maps(plan, per_core, new2old, inputs):
    n, np_, shard, h = plan.n, plan.np_, plan.shard, plan.h
    xsrc = np.asarray(inputs["x"], dtype=np.float32)
    xp = np.zeros((np_, h), dtype=np.float32)
    valid = new2old < n
    xp[valid] = xsrc[new2old[valid]]

    base = {
        "Wo": np.asarray(inputs["Wo"], np.float16),
        "bo": np.tile(np.asarray(inputs["bo"], np.float32).reshape(1, -1),
                      (P, 1)),
    }
    for l in range(plan.n_layers):
        W = np.asarray(inputs[f"W{l}"], np.float32)
        a_s = np.asarray(inputs[f"as{l}"], np.float32)
        a_d = np.asarray(inputs[f"ad{l}"], np.float32)
        Waug = np.concatenate([W, (W @ a_s)[:, None], (W @ a_d)[:, None]],
                              axis=1)
        base[f"Waug{l}"] = Waug.astype(np.float16)
        base[f"B{l}"] = np.tile(
            np.asarray(inputs[f"b{l}"], np.float32).reshape(1, -1), (P, 1))
    in_maps = []
    for c in range(NC):
        m = dict(base)
        xcs = xp[c * shard:(c + 1) * shard]
        m["xT"] = np.ascontiguousarray(xcs.T.astype(np.float16))
        m.update(per_core[c])
        in_maps.append(m)
    return in_maps


_CACHE = {}


def run_gat(inputs, n, h, c_out, **spmd_kwargs):
    edge_index = np.asarray(inputs["edge_index"])
    key = (n, h, c_out, edge_index.shape[1])
    if key not in _CACHE:
        plan = Plan(n, h, c_out)
        per_core, new2old = prep(plan, edge_index)
        nc = build(plan)
        _CACHE[key] = (plan, per_core, new2old, nc)
    plan, per_core, new2old, nc = _CACHE[key]

    in_maps = _make_in_maps(plan, per_core, new2old, inputs)
    res = run_bass_kernel_spmd(nc, in_maps, core_ids=list(range(NC)),
                               **spmd_kwargs)
    shards = [res.results[c]["out"] for c in range(NC)]
    full = np.concatenate(shards, axis=0)
    outp = np.empty((plan.n, plan.c_out), dtype=np.float32)
    valid = new2old < plan.n
    outp[new2old[valid]] = full[valid]
    return outp, res


def kernel(**inputs) -> np.ndarray:
    outp, _ = run_gat(inputs, N_FULL, H_DIM, C_OUT)
    return outp
